# revision 16
# baseline (speedup 1.0000x reference)
"""GCN layer kernel for Trainium2 (8 NeuronCores, SPMD).

out = relu((H + scatter_add(H[src], dst)) @ W)

Sharding: nodes (dst) partitioned across 8 cores (N padded 100000 -> 100352 =
784 blocks of 128; 98 blocks/core). Edge messages H[src] are gathered into a
per-destination-block slot layout during input sharding (fp8 e3m4); this
runtime exposes no working device-side indexed-DMA path (custom GPSIMD ucode
libraries unavailable; vector dynamic DGE offsets broken), so the gather is
part of the host-side shard step.

Scatter-add without per-tile mask generation: within each 128-node block,
nodes are ranked by in-degree (host-side permutation) and every rank r is
padded to a fleet-wide slot run L[r] (sum L = T*128). The per-tile scatter
matrix ("staircase": slot -> rank column) is then identical for every block
and core, so it is shipped once as a small input and loaded as the PE
stationary operand. Blocks are processed in groups of 4 so each stage-1
matmul streams a 512-wide moving operand (4 blocks x 128 features) against
one stair tile -- amortizing LDWEIGHTS and per-matmul overhead -- and
accumulates into one full PSUM bank. The host un-permutes the 128 output
rows of each block after download.

Device per group g (4 dst blocks):
  psum[n, bf]  = sum_t stair_t^T @ msgs_(g,t)    (e3m4 matmuls, f32 accum)
  per block b:
    xt[n, f]   = bf16(psum[:, b] + ht_(g,b))     (DVE add, H bf16 exact)
    xtT[f, n]  = PE-transpose(xt) -> PSUM -> SBUF copy
    out[n, :]  = relu(xtT^T @ W)                 (PE + ACT relu)
"""
import numpy as np
import ml_dtypes

import concourse.bacc as bacc
import concourse.mybir as mybir
from concourse.tile import TileContext
from concourse.bass_utils import run_bass_kernel_spmd

N = 100000
D_IN = 128
D_OUT = 256
N_CORES = 8
N_PAD = 100352
NODES_PER_CORE = N_PAD // N_CORES        # 12544
BLOCKS_PER_CORE = NODES_PER_CORE // 128  # 98
GB = 4                                   # max dst blocks per group (stage-1 moving width)
# small leading groups so the first matmul waits on ~0.4MB, not 1.1MB
GROUP_SIZES = [1, 1] + [GB] * ((BLOCKS_PER_CORE - 2) // GB)  # 1+1+24*4 = 98
assert sum(GROUP_SIZES) == BLOCKS_PER_CORE
N_GROUPS = len(GROUP_SIZES)

bf16 = ml_dtypes.bfloat16
f8e3 = ml_dtypes.float8_e3m4


def build_program(T: int):
    ntiles = BLOCKS_PER_CORE * T         # flat (group, t, block-in-group) order

    nc = bacc.Bacc("TRN2", target_bir_lowering=False)
    msgs_d = nc.declare_dram_parameter("msgs", [128, ntiles, D_IN], mybir.dt.float8e3, isOutput=False)
    ht_d = nc.declare_dram_parameter("ht", [128, BLOCKS_PER_CORE, D_IN], mybir.dt.bfloat16, isOutput=False)
    stair_d = nc.declare_dram_parameter("stair", [128, T, 128], mybir.dt.float8e3, isOutput=False)
    wmat = nc.declare_dram_parameter("wmat", [D_IN, D_OUT], mybir.dt.bfloat16, isOutput=False)
    ident_d = nc.declare_dram_parameter("ident", [128, 128], mybir.dt.bfloat16, isOutput=False)
    # partition-major output: [n-in-block, block, d] -> large DMA descriptors
    out = nc.declare_dram_parameter("out", [128, BLOCKS_PER_CORE, D_OUT], mybir.dt.bfloat16, isOutput=True)

    with TileContext(nc) as tc:
        with (
            tc.tile_pool(name="const", bufs=1) as constp,
            tc.tile_pool(name="msgs", bufs=8) as msgsp,
            tc.tile_pool(name="msgs_s", bufs=3) as msgssp,
            tc.tile_pool(name="xt", bufs=6) as xtp,
            tc.tile_pool(name="xtT", bufs=6) as xtTp,
            tc.tile_pool(name="outp", bufs=6) as outp,
            tc.tile_pool(name="psg", bufs=3, space="PSUM") as psgp,
            tc.tile_pool(name="pstr", bufs=2, space="PSUM") as pstrp,
            tc.tile_pool(name="ps2", bufs=3, space="PSUM") as ps2p,
        ):
            stair_t = constp.tile([128, T, 128], mybir.dt.float8e3)
            nc.sync.dma_start(out=stair_t[:, :, :], in_=stair_d[:, :, :])
            w_t = constp.tile([D_IN, D_OUT], mybir.dt.bfloat16)
            nc.sync.dma_start(out=w_t[:, :], in_=wmat[:, :])
            ident_t = constp.tile([128, 128], mybir.dt.bfloat16)
            nc.sync.dma_start(out=ident_t[:, :], in_=ident_d[:, :])
            # ht is loaded in chunks interleaved with the first groups' msgs
            # DMAs so the big transfer doesn't delay the first matmuls (queues
            # are FIFO: anything issued before msgs g0 lands first).
            ht_t = constp.tile([128, BLOCKS_PER_CORE, D_IN], mybir.dt.bfloat16)
            HT_CHUNKS = 8
            hc_sz = (BLOCKS_PER_CORE + HT_CHUNKS - 1) // HT_CHUNKS  # 13

            blk0 = 0
            tile_off = 0
            for g, gsz in enumerate(GROUP_SIZES):
                pool = msgsp if gsz == GB else msgssp
                msgs_t = pool.tile([128, T, gsz, D_IN], mybir.dt.float8e3,
                                   tag=f"msgs{gsz}")
                nc.sync.dma_start(
                    out=msgs_t[:, :, :, :],
                    in_=msgs_d[:, tile_off : tile_off + T * gsz, :],
                )
                if g < HT_CHUNKS:
                    c0 = g * hc_sz
                    c1 = min(BLOCKS_PER_CORE, c0 + hc_sz)
                    nc.sync.dma_start(out=ht_t[:, c0:c1, :], in_=ht_d[:, c0:c1, :])
                psum_g = psgp.tile([128, GB * D_IN], mybir.dt.float32, tag="psg")
                for t in range(T):
                    nc.tensor.matmul(
                        out=psum_g[:, : gsz * D_IN],
                        lhsT=stair_t[:, t, :],
                        rhs=msgs_t[:, t, :, :],
                        start=(t == 0), stop=(t == T - 1),
                    )
                out_t = outp.tile([128, GB, D_OUT], mybir.dt.bfloat16, tag="out")
                for b in range(gsz):
                    xt_sb = xtp.tile([128, 128], mybir.dt.bfloat16, tag="xt")
                    nc.vector.tensor_tensor(
                        out=xt_sb[:, :],
                        in0=psum_g[:, b * D_IN : (b + 1) * D_IN],
                        in1=ht_t[:, blk0 + b, :],
                        op=mybir.AluOpType.add,
                    )
                    ps_tr = pstrp.tile([128, 128], mybir.dt.bfloat16, tag="pstr")
                    nc.tensor.transpose(out=ps_tr[:, :], in_=xt_sb[:, :], identity=ident_t[:, :])
                    xtT_sb = xtTp.tile([128, 128], mybir.dt.bfloat16, tag="xtT")
                    nc.vector.tensor_copy(out=xtT_sb[:, :], in_=ps_tr[:, :])
                    psum2 = ps2p.tile([128, D_OUT], mybir.dt.float32, tag="ps2")
                    nc.tensor.matmul(out=psum2[:, :], lhsT=xtT_sb[:, :], rhs=w_t[:, :],
                                     start=True, stop=True)
                    nc.scalar.activation(out=out_t[:, b, :], in_=psum2[:, :],
                                         func=mybir.ActivationFunctionType.Relu)
                nc.scalar.dma_start(
                    out=out[:, blk0 : blk0 + gsz, :], in_=out_t[:, :gsz, :]
                )
                blk0 += gsz
                tile_off += T * gsz
    nc.finalize()
    return nc


def preprocess(H, edge_index, W):
    src = np.asarray(edge_index[0], dtype=np.int64)
    dst = np.asarray(edge_index[1], dtype=np.int64)
    H = np.asarray(H, dtype=np.float32)
    W = np.asarray(W, dtype=np.float32)
    E = len(src)

    nblk = N_PAD // 128                                   # 784
    deg = np.bincount(dst, minlength=N_PAD)

    # Global degree-balanced node->(block, rank) assignment: sort all nodes by
    # degree (desc) and deal round-robin, so every block sees nearly the same
    # degree profile and the fleet-wide per-rank run lengths L[r] stay tight.
    g_order = np.argsort(-deg, kind="stable")             # node ids by global degree rank
    g_rank = np.empty(N_PAD, dtype=np.int64)
    g_rank[g_order] = np.arange(N_PAD)
    node_block = g_rank % nblk
    node_rank_in_block = g_rank // nblk
    node_pos = node_block * 128 + node_rank_in_block      # device row of each node
    perm_full = np.empty(N_PAD, dtype=np.int64)
    perm_full[node_pos] = np.arange(N_PAD)

    ranked_deg = deg[perm_full.reshape(nblk, 128)]        # [nblk, 128]
    L = ranked_deg.max(axis=0).astype(np.int64)           # fleet-wide run length per rank
    T = int(np.ceil(max(L.sum(), 1) / 128))
    L[-1] += T * 128 - L.sum()                            # absorb padding in the last rank
    cum = np.concatenate([[0], np.cumsum(L)]).astype(np.int64)  # [129]

    # staircase constants: slot s=t*128+p -> rank column r where cum[r]<=s<cum[r+1]
    slot_rank = np.searchsorted(cum, np.arange(T * 128), side="right") - 1
    stair = np.zeros((T * 128, 128), dtype=f8e3)
    stair[np.arange(T * 128), slot_rank] = 1.0
    stair = np.ascontiguousarray(
        stair.reshape(T, 128, 128).transpose(1, 0, 2)     # [p, t, n]
    )

    # per-edge slot: dst node -> (block, rank) via the dealt assignment
    dst_pos = node_pos[dst]                               # device row of each edge's dst
    order = np.argsort(dst_pos, kind="stable")            # group edges by device row
    sorted_pos = dst_pos[order]
    starts = np.searchsorted(sorted_pos, np.arange(N_PAD))
    k_within = np.arange(E) - starts[sorted_pos]          # edge index within its dst
    blk_of_edge = sorted_pos // 128
    r_of_edge = sorted_pos % 128
    slot_in_block = cum[r_of_edge] + k_within             # in [0, T*128)
    # flat slot in the grouped (g, t, b, p) device layout, per-core local block
    lblk = blk_of_edge % BLOCKS_PER_CORE
    # per-local-block group id / index-in-group / group tile offset
    blk_group = np.empty(BLOCKS_PER_CORE, dtype=np.int64)
    blk_in_group = np.empty(BLOCKS_PER_CORE, dtype=np.int64)
    group_tile_off = np.empty(len(GROUP_SIZES), dtype=np.int64)
    off = 0
    b0 = 0
    for gi, gsz in enumerate(GROUP_SIZES):
        group_tile_off[gi] = off
        blk_group[b0 : b0 + gsz] = gi
        blk_in_group[b0 : b0 + gsz] = np.arange(gsz)
        off += T * gsz
        b0 += gsz
    g_of_edge = blk_group[lblk]
    b_of_edge = blk_in_group[lblk]
    gsz_of_edge = np.asarray(GROUP_SIZES, dtype=np.int64)[g_of_edge]
    t_of_edge = slot_in_block // 128
    p_of_edge = slot_in_block % 128
    tile_of_edge = group_tile_off[g_of_edge] + t_of_edge * gsz_of_edge + b_of_edge
    slot_global = tile_of_edge * 128 + p_of_edge

    H_pad = np.zeros((N_PAD, D_IN), dtype=np.float32)
    H_pad[:N] = H
    H_8 = H_pad.astype(f8e3)
    wmat = W.astype(bf16)
    ident = np.eye(128, dtype=bf16)

    ntiles = BLOCKS_PER_CORE * T
    e_src = src[order]
    in_maps = []
    for c_id in range(N_CORES):
        lo = np.searchsorted(sorted_pos, c_id * NODES_PER_CORE)
        hi = np.searchsorted(sorted_pos, (c_id + 1) * NODES_PER_CORE)
        s = slot_global[lo:hi]
        msgs = np.zeros((ntiles * 128, D_IN), dtype=f8e3)
        msgs[s] = H_8[e_src[lo:hi]]
        msgs = np.ascontiguousarray(
            msgs.reshape(ntiles, 128, D_IN).transpose(1, 0, 2)
        )
        # ht rows follow the device layout: [n-in-block, block, f]
        nodes = perm_full[c_id * NODES_PER_CORE : (c_id + 1) * NODES_PER_CORE]
        ht_arr = np.ascontiguousarray(
            H_pad[nodes].reshape(BLOCKS_PER_CORE, 128, D_IN)
            .transpose(1, 0, 2).astype(bf16)
        )
        in_maps.append({
            "msgs": msgs,
            "ht": ht_arr,
            "stair": stair,
            "wmat": wmat,
            "ident": ident,
        })
    return in_maps, T, perm_full


_PROGRAM_CACHE = {}


def kernel(H, edge_index, W):
    in_maps, T, perm_full = preprocess(H, edge_index, W)
    nc = _PROGRAM_CACHE.get(T)
    if nc is None:
        nc = build_program(T)
        _PROGRAM_CACHE[T] = nc
    res = run_bass_kernel_spmd(nc, in_maps, list(range(N_CORES)))
    out = np.concatenate(
        [np.asarray(res.results[i]["out"]).transpose(1, 0, 2)
         .reshape(NODES_PER_CORE, D_OUT) for i in range(N_CORES)],
        axis=0).astype(np.float32)
    # un-permute: device row p holds node perm_full[p]
    out_full = np.empty_like(out)
    out_full[perm_full] = out
    return np.ascontiguousarray(out_full[:N])


# revision 22
# speedup vs baseline: 1.0380x; 1.0380x over previous
"""GCN layer kernel for Trainium2 (8 NeuronCores, SPMD).

out = relu((H + scatter_add(H[src], dst)) @ W)

Sharding: nodes (dst) partitioned across 8 cores (N padded 100000 -> 100352 =
784 blocks of 128; 98 blocks/core). Edge messages H[src] are gathered into a
per-destination-block slot layout during input sharding (fp8 e3m4); this
runtime exposes no working device-side indexed-DMA path (custom GPSIMD ucode
libraries unavailable; vector dynamic DGE offsets broken), so the gather is
part of the host-side shard step.

Scatter-add without per-tile mask generation: within each 128-node block,
nodes are ranked by in-degree (host-side permutation) and every rank r is
padded to a fleet-wide slot run L[r] (sum L = T*128). The per-tile scatter
matrix ("staircase": slot -> rank column) is then identical for every block
and core, so it is shipped once as a small input and loaded as the PE
stationary operand. Blocks are processed in groups of 4 so each stage-1
matmul streams a 512-wide moving operand (4 blocks x 128 features) against
one stair tile -- amortizing LDWEIGHTS and per-matmul overhead -- and
accumulates into one full PSUM bank. The host un-permutes the 128 output
rows of each block after download.

Device per group g (4 dst blocks):
  psum[n, bf]  = sum_t stair_t^T @ msgs_(g,t)    (e3m4 matmuls, f32 accum)
  per block b:
    xt[n, f]   = bf16(psum[:, b] + ht_(g,b))     (DVE add, H bf16 exact)
    xtT[f, n]  = PE-transpose(xt) -> PSUM -> SBUF copy
    out[n, :]  = relu(xtT^T @ W)                 (PE + ACT relu)
"""
import numpy as np
import ml_dtypes

import concourse.bacc as bacc
import concourse.mybir as mybir
from concourse.tile import TileContext
from concourse.bass_utils import run_bass_kernel_spmd

N = 100000
D_IN = 128
D_OUT = 256
N_CORES = 8
N_PAD = 100352
NODES_PER_CORE = N_PAD // N_CORES        # 12544
BLOCKS_PER_CORE = NODES_PER_CORE // 128  # 98
GB = 4                                   # max dst blocks per group (stage-1 moving width)
# small leading groups so the first matmul waits on ~0.4MB, not 1.1MB
GROUP_SIZES = [1, 1] + [GB] * ((BLOCKS_PER_CORE - 2) // GB)  # 1+1+24*4 = 98
assert sum(GROUP_SIZES) == BLOCKS_PER_CORE
N_GROUPS = len(GROUP_SIZES)

bf16 = ml_dtypes.bfloat16
f8e3 = ml_dtypes.float8_e3m4


def build_program(T: int):
    ntiles = BLOCKS_PER_CORE * T         # flat (group, t, block-in-group) order

    nc = bacc.Bacc("TRN2", target_bir_lowering=False)
    msgs_d = nc.declare_dram_parameter("msgs", [128, ntiles, D_IN], mybir.dt.float8e3, isOutput=False)
    ht_d = nc.declare_dram_parameter("ht", [128, BLOCKS_PER_CORE, D_IN], mybir.dt.bfloat16, isOutput=False)
    stair_d = nc.declare_dram_parameter("stair", [128, T, 128], mybir.dt.float8e3, isOutput=False)
    wmat = nc.declare_dram_parameter("wmat", [D_IN, D_OUT], mybir.dt.bfloat16, isOutput=False)
    ident_d = nc.declare_dram_parameter("ident", [128, 128], mybir.dt.bfloat16, isOutput=False)
    # partition-major output: [n-in-block, block, d] -> large DMA descriptors
    out = nc.declare_dram_parameter("out", [128, BLOCKS_PER_CORE, D_OUT], mybir.dt.bfloat16, isOutput=True)

    with TileContext(nc) as tc:
        with (
            tc.tile_pool(name="const", bufs=1) as constp,
            tc.tile_pool(name="msgs", bufs=8) as msgsp,
            tc.tile_pool(name="msgs_s", bufs=3) as msgssp,
            tc.tile_pool(name="xt", bufs=6) as xtp,
            tc.tile_pool(name="xtT", bufs=6) as xtTp,
            tc.tile_pool(name="outp", bufs=6) as outp,
            tc.tile_pool(name="psg", bufs=3, space="PSUM") as psgp,
            tc.tile_pool(name="pstr", bufs=2, space="PSUM") as pstrp,
            tc.tile_pool(name="ps2", bufs=3, space="PSUM") as ps2p,
        ):
            stair_t = constp.tile([128, T, 128], mybir.dt.float8e3)
            nc.sync.dma_start(out=stair_t[:, :, :], in_=stair_d[:, :, :])
            w_t = constp.tile([D_IN, D_OUT], mybir.dt.bfloat16)
            nc.sync.dma_start(out=w_t[:, :], in_=wmat[:, :])
            ident_t = constp.tile([128, 128], mybir.dt.bfloat16)
            nc.sync.dma_start(out=ident_t[:, :], in_=ident_d[:, :])
            # ht is loaded in chunks interleaved with the first groups' msgs
            # DMAs so the big transfer doesn't delay the first matmuls (queues
            # are FIFO: anything issued before msgs g0 lands first).
            ht_t = constp.tile([128, BLOCKS_PER_CORE, D_IN], mybir.dt.bfloat16)
            HT_CHUNKS = 8
            hc_sz = (BLOCKS_PER_CORE + HT_CHUNKS - 1) // HT_CHUNKS  # 13

            blk0 = 0
            tile_off = 0
            for g, gsz in enumerate(GROUP_SIZES):
                pool = msgsp if gsz == GB else msgssp
                msgs_t = pool.tile([128, T, gsz, D_IN], mybir.dt.float8e3,
                                   tag=f"msgs{gsz}")
                nc.sync.dma_start(
                    out=msgs_t[:, :, :, :],
                    in_=msgs_d[:, tile_off : tile_off + T * gsz, :],
                )
                if g < HT_CHUNKS:
                    c0 = g * hc_sz
                    c1 = min(BLOCKS_PER_CORE, c0 + hc_sz)
                    nc.sync.dma_start(out=ht_t[:, c0:c1, :], in_=ht_d[:, c0:c1, :])
                psum_g = psgp.tile([128, GB * D_IN], mybir.dt.float32, tag="psg")
                for t in range(T):
                    nc.tensor.matmul(
                        out=psum_g[:, : gsz * D_IN],
                        lhsT=stair_t[:, t, :],
                        rhs=msgs_t[:, t, :, :],
                        start=(t == 0), stop=(t == T - 1),
                    )
                out_t = outp.tile([128, GB, D_OUT], mybir.dt.bfloat16, tag="out")
                for b in range(gsz):
                    xt_sb = xtp.tile([128, 128], mybir.dt.bfloat16, tag="xt")
                    nc.vector.tensor_tensor(
                        out=xt_sb[:, :],
                        in0=psum_g[:, b * D_IN : (b + 1) * D_IN],
                        in1=ht_t[:, blk0 + b, :],
                        op=mybir.AluOpType.add,
                    )
                    ps_tr = pstrp.tile([128, 128], mybir.dt.bfloat16, tag="pstr")
                    nc.tensor.transpose(out=ps_tr[:, :], in_=xt_sb[:, :], identity=ident_t[:, :])
                    xtT_sb = xtTp.tile([128, 128], mybir.dt.bfloat16, tag="xtT")
                    nc.vector.tensor_copy(out=xtT_sb[:, :], in_=ps_tr[:, :])
                    psum2 = ps2p.tile([128, D_OUT], mybir.dt.float32, tag="ps2")
                    nc.tensor.matmul(out=psum2[:, :], lhsT=xtT_sb[:, :], rhs=w_t[:, :],
                                     start=True, stop=True)
                    nc.scalar.activation(out=out_t[:, b, :], in_=psum2[:, :],
                                         func=mybir.ActivationFunctionType.Copy)
                nc.scalar.dma_start(
                    out=out[:, blk0 : blk0 + gsz, :], in_=out_t[:, :gsz, :]
                )
                blk0 += gsz
                tile_off += T * gsz
    nc.finalize()
    return nc


def preprocess(H, edge_index, W):
    src = np.asarray(edge_index[0], dtype=np.int64)
    dst = np.asarray(edge_index[1], dtype=np.int64)
    H = np.asarray(H, dtype=np.float32)
    W = np.asarray(W, dtype=np.float32)
    E = len(src)

    nblk = N_PAD // 128                                   # 784
    deg = np.bincount(dst, minlength=N_PAD)

    # Global degree-balanced node->(block, rank) assignment: sort all nodes by
    # degree (desc) and deal round-robin, so every block sees nearly the same
    # degree profile and the fleet-wide per-rank run lengths L[r] stay tight.
    g_order = np.argsort(-deg, kind="stable")             # node ids by global degree rank
    g_rank = np.empty(N_PAD, dtype=np.int64)
    g_rank[g_order] = np.arange(N_PAD)
    node_block = g_rank % nblk
    node_rank_in_block = g_rank // nblk
    node_pos = node_block * 128 + node_rank_in_block      # device row of each node
    perm_full = np.empty(N_PAD, dtype=np.int64)
    perm_full[node_pos] = np.arange(N_PAD)

    ranked_deg = deg[perm_full.reshape(nblk, 128)]        # [nblk, 128]
    L = ranked_deg.max(axis=0).astype(np.int64)           # fleet-wide run length per rank
    # Shave the run profile down to exactly T=16 tiles (2048 slots): each unit
    # decrement of L[r] displaces the over-run edges of every block whose
    # rank-r node has degree > L[r]; those few edges are patched on the host
    # (their contribution is linear, and relu is applied host-side).
    T = 16
    target = T * 128
    excess = int(L.sum() - target)
    if excess < 0:
        L[-1] += -excess
    else:
        for _ in range(excess):
            cost = (ranked_deg >= L[None, :]).sum(axis=0)
            cost[L <= 1] = 1 << 30
            r = int(np.argmin(cost))
            L[r] -= 1
    cum = np.concatenate([[0], np.cumsum(L)]).astype(np.int64)  # [129]

    # staircase constants: slot s=t*128+p -> rank column r where cum[r]<=s<cum[r+1]
    slot_rank = np.searchsorted(cum, np.arange(T * 128), side="right") - 1
    stair = np.zeros((T * 128, 128), dtype=f8e3)
    stair[np.arange(T * 128), slot_rank] = 1.0
    stair = np.ascontiguousarray(
        stair.reshape(T, 128, 128).transpose(1, 0, 2)     # [p, t, n]
    )

    # per-edge slot: dst node -> (block, rank) via the dealt assignment
    dst_pos = node_pos[dst]                               # device row of each edge's dst
    order = np.argsort(dst_pos, kind="stable")            # group edges by device row
    sorted_pos = dst_pos[order]
    starts = np.searchsorted(sorted_pos, np.arange(N_PAD))
    k_within = np.arange(E) - starts[sorted_pos]          # edge index within its dst
    blk_of_edge = sorted_pos // 128
    r_of_edge = sorted_pos % 128
    # edges beyond the (shaved) run length of their rank are patched on host
    keep = k_within < L[r_of_edge]
    e_dst_all = dst[order]
    disp_src = src[order][~keep]
    disp_dst = e_dst_all[~keep]
    slot_in_block = cum[r_of_edge] + k_within             # in [0, T*128) for kept
    # flat slot in the grouped (g, t, b, p) device layout, per-core local block
    lblk = blk_of_edge % BLOCKS_PER_CORE
    # per-local-block group id / index-in-group / group tile offset
    blk_group = np.empty(BLOCKS_PER_CORE, dtype=np.int64)
    blk_in_group = np.empty(BLOCKS_PER_CORE, dtype=np.int64)
    group_tile_off = np.empty(len(GROUP_SIZES), dtype=np.int64)
    off = 0
    b0 = 0
    for gi, gsz in enumerate(GROUP_SIZES):
        group_tile_off[gi] = off
        blk_group[b0 : b0 + gsz] = gi
        blk_in_group[b0 : b0 + gsz] = np.arange(gsz)
        off += T * gsz
        b0 += gsz
    g_of_edge = blk_group[lblk]
    b_of_edge = blk_in_group[lblk]
    gsz_of_edge = np.asarray(GROUP_SIZES, dtype=np.int64)[g_of_edge]
    t_of_edge = slot_in_block // 128
    p_of_edge = slot_in_block % 128
    tile_of_edge = group_tile_off[g_of_edge] + t_of_edge * gsz_of_edge + b_of_edge
    slot_global = tile_of_edge * 128 + p_of_edge

    H_pad = np.zeros((N_PAD, D_IN), dtype=np.float32)
    H_pad[:N] = H
    H_8 = H_pad.astype(f8e3)
    wmat = W.astype(bf16)
    ident = np.eye(128, dtype=bf16)

    ntiles = BLOCKS_PER_CORE * T
    e_src = src[order]
    in_maps = []
    for c_id in range(N_CORES):
        lo = np.searchsorted(sorted_pos, c_id * NODES_PER_CORE)
        hi = np.searchsorted(sorted_pos, (c_id + 1) * NODES_PER_CORE)
        sel = keep[lo:hi]
        s = slot_global[lo:hi][sel]
        msgs = np.zeros((ntiles * 128, D_IN), dtype=f8e3)
        msgs[s] = H_8[e_src[lo:hi][sel]]
        msgs = np.ascontiguousarray(
            msgs.reshape(ntiles, 128, D_IN).transpose(1, 0, 2)
        )
        # ht rows follow the device layout: [n-in-block, block, f]
        nodes = perm_full[c_id * NODES_PER_CORE : (c_id + 1) * NODES_PER_CORE]
        ht_arr = np.ascontiguousarray(
            H_pad[nodes].reshape(BLOCKS_PER_CORE, 128, D_IN)
            .transpose(1, 0, 2).astype(bf16)
        )
        in_maps.append({
            "msgs": msgs,
            "ht": ht_arr,
            "stair": stair,
            "wmat": wmat,
            "ident": ident,
        })
    return in_maps, T, perm_full, (disp_src, disp_dst)


_PROGRAM_CACHE = {}


def kernel(H, edge_index, W):
    in_maps, T, perm_full, disp = preprocess(H, edge_index, W)
    nc = _PROGRAM_CACHE.get(T)
    if nc is None:
        nc = build_program(T)
        _PROGRAM_CACHE[T] = nc
    res = run_bass_kernel_spmd(nc, in_maps, list(range(N_CORES)))
    out = np.concatenate(
        [np.asarray(res.results[i]["out"]).transpose(1, 0, 2)
         .reshape(NODES_PER_CORE, D_OUT) for i in range(N_CORES)],
        axis=0).astype(np.float32)
    # un-permute: device row p holds node perm_full[p] (pre-relu values)
    out_full = np.empty_like(out)
    out_full[perm_full] = out
    # host patch: displaced edges' contribution (linear, so it can be added
    # after download), then the deferred relu
    disp_src, disp_dst = disp
    if len(disp_src):
        Wb = np.asarray(W, np.float32).astype(bf16).astype(np.float32)
        H32 = np.asarray(H, np.float32)
        ud, inv = np.unique(disp_dst, return_inverse=True)
        acc = np.zeros((len(ud), D_IN), np.float32)
        np.add.at(acc, inv, H32[disp_src])
        out_full[ud] += acc @ Wb
    np.maximum(out_full, 0.0, out=out_full)
    return np.ascontiguousarray(out_full[:N])


# revision 28
# speedup vs baseline: 1.0418x; 1.0037x over previous
"""GCN layer kernel for Trainium2 (8 NeuronCores, SPMD).

out = relu((H + scatter_add(H[src], dst)) @ W)

Sharding: nodes (dst) partitioned across 8 cores (N padded 100000 -> 100352 =
784 blocks of 128; 98 blocks/core). Edge messages H[src] are gathered into a
per-destination-block slot layout during input sharding (fp8 e3m4); this
runtime exposes no working device-side indexed-DMA path (custom GPSIMD ucode
libraries unavailable; vector dynamic DGE offsets broken), so the gather is
part of the host-side shard step.

Scatter-add without per-tile mask generation: within each 128-node block,
nodes are ranked by in-degree (host-side permutation) and every rank r is
padded to a fleet-wide slot run L[r] (sum L = T*128). The per-tile scatter
matrix ("staircase": slot -> rank column) is then identical for every block
and core, so it is shipped once as a small input and loaded as the PE
stationary operand. Blocks are processed in groups of 4 so each stage-1
matmul streams a 512-wide moving operand (4 blocks x 128 features) against
one stair tile -- amortizing LDWEIGHTS and per-matmul overhead -- and
accumulates into one full PSUM bank. The host un-permutes the 128 output
rows of each block after download.

Device per group g (4 dst blocks):
  psum[n, bf]  = sum_t stair_t^T @ msgs_(g,t)    (e3m4 matmuls, f32 accum)
  per block b:
    xt[n, f]   = bf16(psum[:, b] + ht_(g,b))     (DVE add, H bf16 exact)
    xtT[f, n]  = PE-transpose(xt) -> PSUM -> SBUF copy
    out[n, :]  = relu(xtT^T @ W)                 (PE + ACT relu)
"""
import numpy as np
import ml_dtypes

import concourse.bacc as bacc
import concourse.mybir as mybir
from concourse.tile import TileContext
from concourse.bass_utils import run_bass_kernel_spmd

N = 100000
D_IN = 128
D_OUT = 256
N_CORES = 8
N_PAD = 100352
NODES_PER_CORE = N_PAD // N_CORES        # 12544
BLOCKS_PER_CORE = NODES_PER_CORE // 128  # 98
GB = 4                                   # max dst blocks per group (stage-1 moving width)
# small leading groups so the first matmul waits on ~0.4MB, not 1.1MB
GROUP_SIZES = [1, 1] + [GB] * 23 + [2, 1, 1]  # small head (fast start) and tail (short drain)
assert sum(GROUP_SIZES) == BLOCKS_PER_CORE
N_GROUPS = len(GROUP_SIZES)

bf16 = ml_dtypes.bfloat16
f8e3 = ml_dtypes.float8_e3m4


def build_program(T: int):
    ntiles = BLOCKS_PER_CORE * T         # flat (group, t, block-in-group) order

    nc = bacc.Bacc("TRN2", target_bir_lowering=False)
    msgs_d = nc.declare_dram_parameter("msgs", [128, ntiles, D_IN], mybir.dt.float8e3, isOutput=False)
    ht_d = nc.declare_dram_parameter("ht", [128, BLOCKS_PER_CORE, D_IN], mybir.dt.bfloat16, isOutput=False)
    stair_d = nc.declare_dram_parameter("stair", [128, T, 128], mybir.dt.float8e3, isOutput=False)
    wmat = nc.declare_dram_parameter("wmat", [D_IN, D_OUT], mybir.dt.bfloat16, isOutput=False)
    ident_d = nc.declare_dram_parameter("ident", [128, 128], mybir.dt.bfloat16, isOutput=False)
    # partition-major output: [n-in-block, block, d] -> large DMA descriptors
    out = nc.declare_dram_parameter("out", [128, BLOCKS_PER_CORE, D_OUT], mybir.dt.bfloat16, isOutput=True)

    with TileContext(nc) as tc:
        with (
            tc.tile_pool(name="const", bufs=1) as constp,
            tc.tile_pool(name="msgs", bufs=8) as msgsp,
            tc.tile_pool(name="msgs_s", bufs=3) as msgssp,
            tc.tile_pool(name="xt", bufs=6) as xtp,
            tc.tile_pool(name="xtT", bufs=6) as xtTp,
            tc.tile_pool(name="outp", bufs=6) as outp,
            tc.tile_pool(name="psg", bufs=3, space="PSUM") as psgp,
            tc.tile_pool(name="pstr", bufs=2, space="PSUM") as pstrp,
            tc.tile_pool(name="ps2", bufs=3, space="PSUM") as ps2p,
        ):
            # only stair gates the first matmul; everything else is issued
            # behind the first msgs groups (queues are FIFO)
            stair_t = constp.tile([128, T, 128], mybir.dt.float8e3)
            nc.sync.dma_start(out=stair_t[:, :, :], in_=stair_d[:, :, :])
            w_t = constp.tile([D_IN, D_OUT], mybir.dt.bfloat16)
            ident_t = constp.tile([128, 128], mybir.dt.bfloat16)
            ht_t = constp.tile([128, BLOCKS_PER_CORE, D_IN], mybir.dt.bfloat16)
            HT_CHUNKS = 8
            hc_sz = (BLOCKS_PER_CORE + HT_CHUNKS - 1) // HT_CHUNKS  # 13

            blk0 = 0
            tile_off = 0
            for g, gsz in enumerate(GROUP_SIZES):
                pool = msgsp if gsz == GB else msgssp
                msgs_t = pool.tile([128, T, gsz, D_IN], mybir.dt.float8e3,
                                   tag=f"msgs{gsz}")
                nc.sync.dma_start(
                    out=msgs_t[:, :, :, :],
                    in_=msgs_d[:, tile_off : tile_off + T * gsz, :],
                )
                if g == 0:
                    nc.sync.dma_start(out=w_t[:, :], in_=wmat[:, :])
                    nc.sync.dma_start(out=ident_t[:, :], in_=ident_d[:, :])
                # chunk g covers blocks [13g, 13g+13) -- always emitted ahead
                # of its readers (group g reads blocks <= 4g+3 < 13g+13), and
                # writes must be emitted before reads (IR is sequential by
                # emission order).
                if g < HT_CHUNKS:
                    c0 = g * hc_sz
                    c1 = min(BLOCKS_PER_CORE, c0 + hc_sz)
                    nc.sync.dma_start(out=ht_t[:, c0:c1, :], in_=ht_d[:, c0:c1, :])
                psum_g = psgp.tile([128, GB * D_IN], mybir.dt.float32, tag="psg")
                for t in range(T):
                    nc.tensor.matmul(
                        out=psum_g[:, : gsz * D_IN],
                        lhsT=stair_t[:, t, :],
                        rhs=msgs_t[:, t, :, :],
                        start=(t == 0), stop=(t == T - 1),
                    )
                out_t = outp.tile([128, GB, D_OUT], mybir.dt.bfloat16, tag="out")
                for b in range(gsz):
                    xt_sb = xtp.tile([128, 128], mybir.dt.bfloat16, tag="xt")
                    nc.vector.tensor_tensor(
                        out=xt_sb[:, :],
                        in0=psum_g[:, b * D_IN : (b + 1) * D_IN],
                        in1=ht_t[:, blk0 + b, :],
                        op=mybir.AluOpType.add,
                    )
                    ps_tr = pstrp.tile([128, 128], mybir.dt.bfloat16, tag="pstr")
                    nc.tensor.transpose(out=ps_tr[:, :], in_=xt_sb[:, :], identity=ident_t[:, :])
                    xtT_sb = xtTp.tile([128, 128], mybir.dt.bfloat16, tag="xtT")
                    nc.vector.tensor_copy(out=xtT_sb[:, :], in_=ps_tr[:, :])
                    psum2 = ps2p.tile([128, D_OUT], mybir.dt.float32, tag="ps2")
                    nc.tensor.matmul(out=psum2[:, :], lhsT=xtT_sb[:, :], rhs=w_t[:, :],
                                     start=True, stop=True)
                    nc.scalar.activation(out=out_t[:, b, :], in_=psum2[:, :],
                                         func=mybir.ActivationFunctionType.Copy)
                nc.scalar.dma_start(
                    out=out[:, blk0 : blk0 + gsz, :], in_=out_t[:, :gsz, :]
                )
                blk0 += gsz
                tile_off += T * gsz
    nc.finalize()
    return nc


def preprocess(H, edge_index, W):
    src = np.asarray(edge_index[0], dtype=np.int64)
    dst = np.asarray(edge_index[1], dtype=np.int64)
    H = np.asarray(H, dtype=np.float32)
    W = np.asarray(W, dtype=np.float32)
    E = len(src)

    nblk = N_PAD // 128                                   # 784
    deg = np.bincount(dst, minlength=N_PAD)

    # Global degree-balanced node->(block, rank) assignment: sort all nodes by
    # degree (desc) and deal round-robin, so every block sees nearly the same
    # degree profile and the fleet-wide per-rank run lengths L[r] stay tight.
    g_order = np.argsort(-deg, kind="stable")             # node ids by global degree rank
    g_rank = np.empty(N_PAD, dtype=np.int64)
    g_rank[g_order] = np.arange(N_PAD)
    node_block = g_rank % nblk
    node_rank_in_block = g_rank // nblk
    node_pos = node_block * 128 + node_rank_in_block      # device row of each node
    perm_full = np.empty(N_PAD, dtype=np.int64)
    perm_full[node_pos] = np.arange(N_PAD)

    ranked_deg = deg[perm_full.reshape(nblk, 128)]        # [nblk, 128]
    L = ranked_deg.max(axis=0).astype(np.int64)           # fleet-wide run length per rank
    # Shave the run profile down to exactly T=16 tiles (2048 slots): each unit
    # decrement of L[r] displaces the over-run edges of every block whose
    # rank-r node has degree > L[r]; those few edges are patched on the host
    # (their contribution is linear, and relu is applied host-side).
    T = 16
    target = T * 128
    excess = int(L.sum() - target)
    if excess < 0:
        L[-1] += -excess
    else:
        for _ in range(excess):
            cost = (ranked_deg >= L[None, :]).sum(axis=0)
            cost[L <= 1] = 1 << 30
            r = int(np.argmin(cost))
            L[r] -= 1
    cum = np.concatenate([[0], np.cumsum(L)]).astype(np.int64)  # [129]

    # staircase constants: slot s=t*128+p -> rank column r where cum[r]<=s<cum[r+1]
    slot_rank = np.searchsorted(cum, np.arange(T * 128), side="right") - 1
    stair = np.zeros((T * 128, 128), dtype=f8e3)
    stair[np.arange(T * 128), slot_rank] = 1.0
    stair = np.ascontiguousarray(
        stair.reshape(T, 128, 128).transpose(1, 0, 2)     # [p, t, n]
    )

    # per-edge slot: dst node -> (block, rank) via the dealt assignment
    dst_pos = node_pos[dst]                               # device row of each edge's dst
    order = np.argsort(dst_pos, kind="stable")            # group edges by device row
    sorted_pos = dst_pos[order]
    starts = np.searchsorted(sorted_pos, np.arange(N_PAD))
    k_within = np.arange(E) - starts[sorted_pos]          # edge index within its dst
    blk_of_edge = sorted_pos // 128
    r_of_edge = sorted_pos % 128
    # edges beyond the (shaved) run length of their rank are patched on host
    keep = k_within < L[r_of_edge]
    e_dst_all = dst[order]
    disp_src = src[order][~keep]
    disp_dst = e_dst_all[~keep]
    slot_in_block = cum[r_of_edge] + k_within             # in [0, T*128) for kept
    # flat slot in the grouped (g, t, b, p) device layout, per-core local block
    lblk = blk_of_edge % BLOCKS_PER_CORE
    # per-local-block group id / index-in-group / group tile offset
    blk_group = np.empty(BLOCKS_PER_CORE, dtype=np.int64)
    blk_in_group = np.empty(BLOCKS_PER_CORE, dtype=np.int64)
    group_tile_off = np.empty(len(GROUP_SIZES), dtype=np.int64)
    off = 0
    b0 = 0
    for gi, gsz in enumerate(GROUP_SIZES):
        group_tile_off[gi] = off
        blk_group[b0 : b0 + gsz] = gi
        blk_in_group[b0 : b0 + gsz] = np.arange(gsz)
        off += T * gsz
        b0 += gsz
    g_of_edge = blk_group[lblk]
    b_of_edge = blk_in_group[lblk]
    gsz_of_edge = np.asarray(GROUP_SIZES, dtype=np.int64)[g_of_edge]
    t_of_edge = slot_in_block // 128
    p_of_edge = slot_in_block % 128
    tile_of_edge = group_tile_off[g_of_edge] + t_of_edge * gsz_of_edge + b_of_edge
    slot_global = tile_of_edge * 128 + p_of_edge

    H_pad = np.zeros((N_PAD, D_IN), dtype=np.float32)
    H_pad[:N] = H
    H_8 = H_pad.astype(f8e3)
    wmat = W.astype(bf16)
    ident = np.eye(128, dtype=bf16)

    ntiles = BLOCKS_PER_CORE * T
    e_src = src[order]
    in_maps = []
    for c_id in range(N_CORES):
        lo = np.searchsorted(sorted_pos, c_id * NODES_PER_CORE)
        hi = np.searchsorted(sorted_pos, (c_id + 1) * NODES_PER_CORE)
        sel = keep[lo:hi]
        s = slot_global[lo:hi][sel]
        msgs = np.zeros((ntiles * 128, D_IN), dtype=f8e3)
        msgs[s] = H_8[e_src[lo:hi][sel]]
        msgs = np.ascontiguousarray(
            msgs.reshape(ntiles, 128, D_IN).transpose(1, 0, 2)
        )
        # ht rows follow the device layout: [n-in-block, block, f]
        nodes = perm_full[c_id * NODES_PER_CORE : (c_id + 1) * NODES_PER_CORE]
        ht_arr = np.ascontiguousarray(
            H_pad[nodes].reshape(BLOCKS_PER_CORE, 128, D_IN)
            .transpose(1, 0, 2).astype(bf16)
        )
        in_maps.append({
            "msgs": msgs,
            "ht": ht_arr,
            "stair": stair,
            "wmat": wmat,
            "ident": ident,
        })
    return in_maps, T, perm_full, (disp_src, disp_dst)


_PROGRAM_CACHE = {}


def kernel(H, edge_index, W):
    in_maps, T, perm_full, disp = preprocess(H, edge_index, W)
    nc = _PROGRAM_CACHE.get(T)
    if nc is None:
        nc = build_program(T)
        _PROGRAM_CACHE[T] = nc
    res = run_bass_kernel_spmd(nc, in_maps, list(range(N_CORES)))
    out = np.concatenate(
        [np.asarray(res.results[i]["out"]).transpose(1, 0, 2)
         .reshape(NODES_PER_CORE, D_OUT) for i in range(N_CORES)],
        axis=0).astype(np.float32)
    # un-permute: device row p holds node perm_full[p] (pre-relu values)
    out_full = np.empty_like(out)
    out_full[perm_full] = out
    # host patch: displaced edges' contribution (linear, so it can be added
    # after download), then the deferred relu
    disp_src, disp_dst = disp
    if len(disp_src):
        Wb = np.asarray(W, np.float32).astype(bf16).astype(np.float32)
        H32 = np.asarray(H, np.float32)
        ud, inv = np.unique(disp_dst, return_inverse=True)
        acc = np.zeros((len(ud), D_IN), np.float32)
        np.add.at(acc, inv, H32[disp_src])
        out_full[ud] += acc @ Wb
    np.maximum(out_full, 0.0, out=out_full)
    return np.ascontiguousarray(out_full[:N])
